# revision 11
# baseline (speedup 1.0000x reference)
"""Linformer attention block on 8 TRN2 NeuronCores, data-parallel over batch.

v3: fp8e4 DoubleRow (0.5 cyc/col) for projection/attention matmuls where
walrus allows it (128-out-partition at offset 0); plain fp8 elsewhere (dots,
attn@v second head). K/V projections reassociated as (proj^T y) @ W. bf16
ones-matmul LN stats (fp32r rejected by the BIR verifier), stats staging on
the Pool engine. Two-phase emission per rep (all LN/projections, then all
attention) so ACT loads each activation table once. Softmax denominators via
zero-padded fp8 DoubleRow ones-matmul into psum rows 0:2. x+pos via resident
pos tile + Pool add (HBM read once). Wo bias folded into pos host-side.

Scale bookkeeping (host pre-scales, device compensates at evictions):
  y8 = 16 y;  wq8 = 512 (Wq dh^-.5);  q8 = 128 q;   pk8/pv8 = 256 proj
  yk8 = 16 yk; wk8/wv8/wo8 = 64 W;    k8 = 32 k;    v8 = 32 v;  ao8 = 32 ao
  dots_psum = 4096 dots  -> exp scale 1/4096
  wo_psum  = 2048 out    -> final evict scale 1/2048
"""

import os
import sys
import types

import numpy as np
import ml_dtypes

try:
    import antenv.axon_hooks  # noqa: F401
except ImportError:
    _shim = types.ModuleType("antenv.axon_hooks")
    _shim.get_axon_ntff_profile_hook = lambda: None
    sys.modules["antenv.axon_hooks"] = _shim

import concourse.bass as bass
import concourse.mybir as mybir
from concourse import bacc
from concourse.tile import TileContext
from concourse.bass_utils import run_bass_kernel_spmd

F32 = mybir.dt.float32
BF16 = mybir.dt.bfloat16
F8 = mybir.dt.float8e4
OP = mybir.AluOpType
AF = mybir.ActivationFunctionType
DR = mybir.MatmulPerfMode.DoubleRow

B, C, HH, WW = 32, 512, 32, 32
N = HH * WW            # 1024
HEADS = 8
DH = C // HEADS        # 64
KLR = 256              # linformer rank
EPS = 1e-5
NCORES = 8
BL = B // NCORES       # 4 batch elems per core
CC = C // 128          # 4 channel chunks
NH = N // 512          # 2 free halves
KC = KLR // 128        # 2 k chunks
NT = N // 128          # 8 token chunks

S_Y, S_WQ, S_Q = 16.0, 512.0, 128.0
S_P, S_YK, S_W, S_K, S_V = 256.0, 16.0, 64.0, 32.0, 32.0


def _rearr(d):
    return d[:].rearrange("(a p) n -> p a n", p=128)


def _build(reps=1):
    nc = bacc.Bacc()
    dp = nc.declare_dram_parameter
    x_d = dp("x", [BL, C, N], F32, isOutput=False)
    posT_d = dp("posT", [C, N], F32, isOutput=False)
    wq_d = dp("wq", [C, C], F8, isOutput=False)
    wk_d = dp("wk", [C, C], F8, isOutput=False)
    wv_d = dp("wv", [C, C], F8, isOutput=False)
    wo_d = dp("wo", [C, C], F8, isOutput=False)
    pk_d = dp("pk", [N, KLR], F8, isOutput=False)
    pv_d = dp("pv", [N, KLR], F8, isOutput=False)
    ident_d = dp("ident", [128, 128], F8, isOutput=False)
    sel2_d = dp("sel2", [2, 128], BF16, isOutput=False)
    e2_d = dp("e2", [128, 2, 2, 128], F8, isOutput=False)
    gcol_d = dp("gcol", [128, CC], F32, isOutput=False)
    lnb16_d = dp("lnb16", [128, CC], F32, isOutput=False)
    out_d = dp("out", [BL, C, N], F32, isOutput=True)

    with TileContext(nc) as tc:
        with (
            tc.tile_pool(name="const", bufs=1) as cp,
            tc.tile_pool(name="work", bufs=2) as wp,
        ):
            posT = cp.tile([128, CC, N], F32)
            nc.sync.dma_start(out=posT, in_=_rearr(posT_d))
            wq = cp.tile([128, CC, C], F8)
            nc.sync.dma_start(out=wq, in_=_rearr(wq_d))
            wk = cp.tile([128, CC, C], F8)
            nc.sync.dma_start(out=wk, in_=_rearr(wk_d))
            wv = cp.tile([128, CC, C], F8)
            nc.sync.dma_start(out=wv, in_=_rearr(wv_d))
            wo = cp.tile([128, CC, C], F8)
            nc.sync.dma_start(out=wo, in_=_rearr(wo_d))
            pk = cp.tile([128, NT, KLR], F8)
            nc.sync.dma_start(out=pk, in_=_rearr(pk_d))
            pv = cp.tile([128, NT, KLR], F8)
            nc.sync.dma_start(out=pv, in_=_rearr(pv_d))
            ident = cp.tile([128, 128], F8)
            nc.sync.dma_start(out=ident, in_=ident_d[:])
            gcol = cp.tile([128, CC], F32)
            nc.sync.dma_start(out=gcol, in_=gcol_d[:])
            lnb16 = cp.tile([128, CC], F32)
            nc.sync.dma_start(out=lnb16, in_=lnb16_d[:])

            onesb = cp.tile([128, 1], BF16)
            nc.vector.memset(onesb, 1.0)
            onesrow = cp.tile([1, 128], BF16)
            nc.vector.memset(onesrow, 1.0)
            sel2 = cp.tile([2, 128], BF16)
            nc.sync.dma_start(out=sel2, in_=sel2_d[:])
            # sums lhsT: e2[:, hp, kc, j] = 1 iff j == hp (zero-padded to 128
            # cols; DoubleRow requires full-width weights)
            e2 = cp.tile([128, 2, 2, 128], F8)
            nc.sync.dma_start(out=e2, in_=e2_d[:])
            epsc = cp.tile([1, 1], F32)
            nc.vector.memset(epsc, EPS)

            c = dict(wq=wq, wk=wk, wv=wv, wo=wo, pk=pk, pv=pv, ident=ident,
                     gcol=gcol, lnb16=lnb16, onesb=onesb, onesrow=onesrow,
                     sel2=sel2, e2=e2, epsc=epsc, posT=posT)
            with nc.allow_low_precision(reason="fp8 attention path"):
                for _rep in range(reps):
                    fronts = []
                    with (tc.tile_pool(name=f"psA{_rep}", bufs=2,
                                       space="PSUM") as ppa,
                          tc.tile_pool(name=f"wA{_rep}", bufs=2) as wpa):
                        for b0 in range(0, BL, 2):
                            fronts.extend(
                                _emit_front_pair(nc, wp, wpa, ppa,
                                                 (b0, b0 + 1), x_d, c))
                    with (tc.tile_pool(name=f"psB{_rep}", bufs=2,
                                       space="PSUM") as ppb,
                          tc.tile_pool(name=f"wB{_rep}", bufs=2) as wpb):
                        _emit_back_all(nc, wp, wpb, ppb, out_d, c, fronts)
    nc.compile()
    return nc


def _emit_front(nc, wp, pp, b, x_d, c):
    """LN + q/k/v projections for batch b. ACT funcs: Sqrt, Relu, Copy."""
    wq, wk, wv = c["wq"], c["wk"], c["wv"]
    pk, pv, ident = c["pk"], c["pv"], c["ident"]
    gcol, lnb16, epsc = c["gcol"], c["lnb16"], c["epsc"]
    onesb, onesrow, posT = c["onesb"], c["onesrow"], c["posT"]

    # ------- s = x + posT' in bf16 (posT' carries Wo bias) -------
    # bf16 s costs ~0.03 abs on the residual (budget 0.15) and lets the LN
    # stats matmuls + LN apply consume it directly with no bf16 staging.
    x = wp.tile([128, CC, N], F32, tag="x", bufs=2)
    nc.sync.dma_start(out=x, in_=x_d[b].rearrange("(a p) n -> p a n", p=128))
    s = wp.tile([128, CC, N], BF16, tag="s", bufs=BL)
    nc.gpsimd.tensor_add(s, x, posT)

    sqs = []
    for cc in range(CC):
        sq = wp.tile([128, N], BF16, tag="sq", bufs=4)
        nc.gpsimd.tensor_mul(sq, s[:, cc, :], s[:, cc, :])
        sqs.append(sq)

    # ---------------- LN statistics ----------------
    minis = wp.tile([1, 2, N], BF16, tag="minis", bufs=2)  # [0]=mean [1]=rstd
    for nh in range(NH):
        nsl = slice(nh * 512, (nh + 1) * 512)
        s1 = pp.tile([1, 512], F32, tag="st", bufs=2)
        for cc in range(CC):
            nc.tensor.matmul(s1, onesb[:], s[:, cc, nsl],
                             start=(cc == 0), stop=(cc == CC - 1))
        s2 = pp.tile([1, 512], F32, tag="st", bufs=2)
        for cc in range(CC):
            nc.tensor.matmul(s2, onesb[:], sqs[cc][:, nsl],
                             start=(cc == 0), stop=(cc == CC - 1))
        mean = minis[0:1, 0, nsl]
        nc.vector.tensor_scalar_mul(mean, s1, 1.0 / C)
        m2 = wp.tile([1, 512], F32, tag="mini2", bufs=4)
        nc.vector.tensor_mul(m2, mean, s1)  # = C * mean^2
        v512 = wp.tile([1, 512], F32, tag="mini2", bufs=4)
        nc.vector.scalar_tensor_tensor(v512, in0=m2, scalar=-1.0, in1=s2,
                                       op0=OP.mult, op1=OP.add)  # C*var
        sd = wp.tile([1, 512], F32, tag="mini2", bufs=4)
        nc.scalar.activation(sd, v512, AF.Sqrt, bias=epsc[:], scale=1.0 / C)
        nc.vector.reciprocal(minis[0:1, 1, nsl], sd)

    # broadcast mean & rstd down 128 partitions in one outer product, then
    # evict to bf16 SBUF so the LN-apply DVE ops run on pure-SBUF operands
    bc = pp.tile([128, 2, N], F32, tag="bc", bufs=1)
    for j in range(2):
        for nh in range(NH):
            nsl = slice(nh * 512, (nh + 1) * 512)
            nc.tensor.matmul(bc[:, j, nsl], onesrow[:], minis[0:1, j, nsl],
                             start=True, stop=True)
    bcs = wp.tile([128, 2, N], BF16, tag="bcs", bufs=2)
    nc.scalar.copy(bcs, bc)

    # ---------------- LN apply + relu -> y8 (fp8, x16) ----------------
    y8 = wp.tile([128, CC, N], F8, tag="y8", bufs=2)
    for cc in range(CC):
        t1 = wp.tile([128, N], BF16, tag="lnt", bufs=2)
        nc.vector.tensor_sub(t1, s[:, cc, :], bcs[:, 0, :])
        t2 = wp.tile([128, N], BF16, tag="lnt2", bufs=2)
        nc.vector.scalar_tensor_tensor(t2, in0=t1, scalar=gcol[:, cc:cc + 1],
                                       in1=bcs[:, 1, :], op0=OP.mult,
                                       op1=OP.mult)
        nc.scalar.activation(y8[:, cc, :], t2, AF.Relu,
                             bias=lnb16[:, cc:cc + 1], scale=S_Y)

    # ---------------- qT [d_part, dc, n] fp8 (x128) ----------------
    q8 = wp.tile([128, CC, N], F8, tag="q8", bufs=BL)
    for dc in range(CC):
        for nh in range(NH):
            nsl = slice(nh * 512, (nh + 1) * 512)
            ps = pp.tile([128, 512], F32, tag="pj", bufs=2)
            for kcp in range(0, CC, 2):
                nc.tensor.matmul(ps, wq[:, kcp:kcp + 2, dc * 128:(dc + 1) * 128],
                                 y8[:, kcp:kcp + 2, nsl], perf_mode=DR,
                                 start=(kcp == 0), stop=(kcp == CC - 2))
            if nh == 0:
                nc.vector.tensor_scalar_mul(q8[:, dc, nsl], ps,
                                            S_Q / (S_Y * S_WQ))
            else:
                nc.scalar.activation(q8[:, dc, nsl], ps, AF.Copy,
                                     scale=S_Q / (S_Y * S_WQ))

    # ---------------- ytok [n_part, t, c] fp8 via PE transpose -------------
    # fp8 transpose writes with element step 2: view the psum bank as
    # [128, 1024, 2] and use component 0.
    ytok = wp.tile([128, NT, C], F8, tag="ytok", bufs=2)
    for half in range(4):
        ps = pp.tile([128, 512], F32, tag="pj", bufs=2)
        ps8 = ps[:].bitcast(F8).rearrange("p (a t) -> p a t", t=2)
        for ti in range(2):
            t = half * 2 + ti
            for kc in range(CC):
                nc.tensor.transpose(
                    ps8[:, ti * 512 + kc * 128: ti * 512 + (kc + 1) * 128, 0],
                    y8[:, kc, t * 128:(t + 1) * 128], ident[:])
        ev = nc.vector if half % 2 == 0 else nc.scalar
        if half % 2 == 0:
            nc.vector.tensor_copy(ytok[:, half * 2:(half + 1) * 2, :],
                                  ps8[:, :, 0])
        else:
            nc.scalar.copy(ytok[:, half * 2:(half + 1) * 2, :], ps8[:, :, 0])

    # ---------------- ybar_k/v^T [c_part, cc, K] fp8 = (proj^T y)^T --------
    ybars = []
    for which, proj in (("k", pk), ("v", pv)):
        yb = wp.tile([128, CC, KLR], F8, tag=f"yb{which}", bufs=2)
        for ccp in range(0, CC, 2):
            ps = pp.tile([128, 512], F32, tag="pj", bufs=2)
            for i in range(2):
                cc = ccp + i
                for tp in range(0, NT, 2):
                    nc.tensor.matmul(
                        ps[:, i * 256:(i + 1) * 256],
                        ytok[:, tp:tp + 2, cc * 128:(cc + 1) * 128],
                        proj[:, tp:tp + 2, :], perf_mode=DR,
                        start=(tp == 0), stop=(tp == NT - 2),
                        skip_group_check=True)
            if which == "k":
                nc.vector.tensor_scalar_mul(yb[:, ccp:ccp + 2, :], ps,
                                            S_YK / (S_Y * S_P))
            else:
                nc.scalar.activation(yb[:, ccp:ccp + 2, :], ps, AF.Copy,
                                     scale=S_YK / (S_Y * S_P))
        ybars.append(yb)
    ybk, ybv = ybars

    # ---------------- ktbf [d_part, dc, K] fp8 = (ybar_k Wk)^T -------------
    ktbf = wp.tile([128, CC, KLR], F8, tag="ktbf", bufs=BL)
    for dcp in range(0, CC, 2):
        ps = pp.tile([128, 512], F32, tag="pj", bufs=2)
        for i in range(2):
            dc = dcp + i
            for kcp in range(0, CC, 2):
                nc.tensor.matmul(ps[:, i * 256:(i + 1) * 256],
                                 wk[:, kcp:kcp + 2, dc * 128:(dc + 1) * 128],
                                 ybk[:, kcp:kcp + 2, :], perf_mode=DR,
                                 start=(kcp == 0), stop=(kcp == CC - 2),
                                 skip_group_check=True)
        nc.scalar.activation(ktbf[:, dcp:dcp + 2, :], ps, AF.Copy,
                             scale=S_K / (S_YK * S_W))

    # ---------------- vsbf [k_part, kc, d] fp8 = ybar_v Wv ----------------
    vsbf = wp.tile([128, KC, C], F8, tag="vsbf", bufs=BL)
    for kc in range(KC):
        ps = pp.tile([128, 512], F32, tag="pj", bufs=2)
        for ccp in range(0, CC, 2):
            nc.tensor.matmul(ps, ybv[:, ccp:ccp + 2, kc * 128:(kc + 1) * 128],
                             wv[:, ccp:ccp + 2, :], perf_mode=DR,
                             start=(ccp == 0), stop=(ccp == CC - 2))
        nc.vector.tensor_scalar_mul(vsbf[:, kc, :], ps, S_V / (S_YK * S_W))

    return dict(s=s, q8=q8, ktbf=ktbf, vsbf=vsbf)


def _emit_back(nc, wp, pp, b, out_d, c, front):
    """Attention + Wo + residual for batch b. ACT funcs: Exp, Copy."""
    wo, sel2, e2 = c["wo"], c["sel2"], c["e2"]
    s, q8, ktbf, vsbf = front["s"], front["q8"], front["ktbf"], front["vsbf"]

    aobf = wp.tile([128, CC, N], F8, tag="aobf", bufs=2)
    for nh in range(NH):
        nsl = slice(nh * 512, (nh + 1) * 512)
        for pr in range(CC):  # head pair (2pr, 2pr+1)
            attn = {}
            sums = pp.tile([128, 512], F32, tag="sm", bufs=1,
                           name=f"sums_{b}_{nh}_{pr}")
            for hp in range(2):
                rsl = slice(hp * 64, (hp + 1) * 64)
                dps = pp.tile([128, 2, 512], F32, tag="dp", bufs=2,
                              name=f"dps_{b}_{nh}_{pr}_{hp}")
                for kc in range(KC):
                    nc.tensor.matmul(dps[:, kc, :],
                                     ktbf[rsl, pr, kc * 128:(kc + 1) * 128],
                                     q8[rsl, pr, nsl], start=True, stop=True)
                at = wp.tile([128, 2, 512], F8, tag="attn", bufs=4,
                             name=f"at_{b}_{nh}_{pr}_{hp}")
                attn[hp] = at
                nc.scalar.activation(at, dps, AF.Exp, scale=1.0 / (S_Q * S_K))
                nc.tensor.matmul(sums, e2[:, hp, :, :], at[:, :, :],
                                 perf_mode=DR, start=(hp == 0), stop=(hp == 1),
                                 skip_group_check=True)
            recip = wp.tile([2, 512], BF16, tag="recip", bufs=2)
            nc.vector.reciprocal(recip, sums[0:2, :])
            rbp = pp.tile([128, 512], F32, tag="av", bufs=3,
                          name=f"rbc_{b}_{nh}_{pr}")
            nc.tensor.matmul(rbp, sel2[:], recip[:], start=True, stop=True)
            rbc = wp.tile([128, 512], BF16, tag="rbc", bufs=2,
                          name=f"rbcs_{b}_{nh}_{pr}")
            nc.scalar.copy(rbc, rbp)
            av = pp.tile([128, 512], F32, tag="av", bufs=3,
                         name=f"av_{b}_{nh}_{pr}")
            # hp0 via DoubleRow at rows 0:64; DR cannot write at partition
            # offset 64, so hp1 uses two plain fp8 matmuls there
            nc.tensor.matmul(av[0:64, :], vsbf[:, :, (2 * pr) * 64:
                                                (2 * pr + 1) * 64],
                             attn[0][:, :, :], perf_mode=DR,
                             start=True, stop=True)
            h1 = 2 * pr + 1
            for kc in range(KC):
                nc.tensor.matmul(av[64:128, :],
                                 vsbf[:, kc, h1 * 64:(h1 + 1) * 64],
                                 attn[1][:, kc, :],
                                 start=(kc == 0), stop=(kc == KC - 1),
                                 tile_position=(0, 64),
                                 skip_group_check=True)
            nc.vector.scalar_tensor_tensor(aobf[:, pr, nsl], in0=av,
                                           scalar=0.0, in1=rbc,
                                           op0=OP.bypass, op1=OP.mult)

    # ---------------- Wo + residual (+bias via posT') -> out ----------------
    for co in range(CC):
        outf = wp.tile([128, N], F32, tag="outf", bufs=2)
        for nh in range(NH):
            nsl = slice(nh * 512, (nh + 1) * 512)
            ps = pp.tile([128, 2, 512], F32, tag="dp", bufs=2,
                         name=f"wo_{b}_{co}_{nh}")
            for pp_ in range(0, CC, 2):
                nc.tensor.matmul(ps[:, 0, :],
                                 wo[:, pp_:pp_ + 2, co * 128:(co + 1) * 128],
                                 aobf[:, pp_:pp_ + 2, nsl], perf_mode=DR,
                                 start=(pp_ == 0), stop=(pp_ == CC - 2))
            nc.vector.scalar_tensor_tensor(outf[:, nsl], in0=ps[:, 0, :],
                                           scalar=1.0 / (S_V * S_W),
                                           in1=s[:, co, nsl],
                                           op0=OP.mult, op1=OP.add)
        nc.sync.dma_start(out=out_d[b, co * 128:(co + 1) * 128, :], in_=outf)


_CACHE = {}


def get_nc(reps=1):
    key = ("nc", reps)
    if key not in _CACHE:
        _CACHE[key] = _build(reps)
    return _CACHE[key]


def _sel2_host():
    sel2 = np.zeros((2, 128), ml_dtypes.bfloat16)
    sel2[0, 0:64] = 1
    sel2[1, 64:128] = 1
    return sel2


def _e2_host():
    e2 = np.zeros((128, 2, 2, 128), ml_dtypes.float8_e4m3)
    e2[:, 0, :, 0] = 1
    e2[:, 1, :, 1] = 1
    return e2


def make_in_maps(inputs):
    f8 = ml_dtypes.float8_e4m3
    x = np.ascontiguousarray(np.asarray(inputs["x"], np.float32)
                             .reshape(B, C, N))
    pos = np.asarray(inputs["pos"], np.float32).reshape(N, C)
    ln_g = np.asarray(inputs["ln_g"], np.float32)
    ln_b = np.asarray(inputs["ln_b"], np.float32)
    bo = np.asarray(inputs["bo"], np.float32)

    posT = np.ascontiguousarray(pos.T) + bo[:, None]  # fold Wo bias into pos

    shared = {
        "posT": posT,
        "wq": (np.asarray(inputs["Wq"], np.float32)
               * (DH ** -0.5) * S_WQ).astype(f8),
        "wk": (np.asarray(inputs["Wk"], np.float32) * S_W).astype(f8),
        "wv": (np.asarray(inputs["Wv"], np.float32) * S_W).astype(f8),
        "wo": (np.asarray(inputs["Wo"], np.float32) * S_W).astype(f8),
        "pk": (np.asarray(inputs["proj_k"], np.float32) * S_P).astype(f8),
        "pv": (np.asarray(inputs["proj_v"], np.float32) * S_P).astype(f8),
        "ident": np.eye(128, dtype=f8),
        "sel2": _sel2_host(),
        "e2": _e2_host(),
        "gcol": np.ascontiguousarray(ln_g.reshape(CC, 128).T),
        "lnb16": np.ascontiguousarray((S_Y * ln_b).reshape(CC, 128).T),
    }
    return [dict(shared, x=np.ascontiguousarray(x[i * BL:(i + 1) * BL]))
            for i in range(NCORES)]


def kernel(**inputs):
    nc = get_nc()
    in_maps = make_in_maps(inputs)
    trace = bool(int(os.environ.get("BASS_KERNEL_TRACE", "0")))
    res = run_bass_kernel_spmd(nc, in_maps, core_ids=list(range(NCORES)),
                               trace=trace)
    kernel.last_result = res
    out = np.concatenate([np.asarray(res.results[i]["out"], np.float32)
                          [None] for i in range(NCORES)], axis=0)
    return np.ascontiguousarray(out.reshape(B, C, HH, WW))
